# revision 34
# baseline (speedup 1.0000x reference)
"""DGCNN segmentation Bass kernel for 8 trn2 NeuronCores.

Sharding: data-parallel over (cloud b, half h): core c = 2*b + h owns
queries [h*2048:(h+1)*2048] of cloud b. Candidates are kept in a
core-relative order (own half first) so the SPMD program is identical on
every core; halves are exchanged per layer with a pairwise AllReduce(add)
(partner = sum - own).

EdgeConv is linearized: W @ [xj - xi; xi] = W1 @ xj + (W2 - W1) @ xi, and
eval-mode BN is folded into the weights on the host. LeakyReLU(0.2) is the
ACT engine's Prelu. Per-query top-20 neighbours come from 3 rounds of
max8/max_index/match_replace on the negative half-distance row; neighbour
feature rows are gathered with per-partition indirect DMA and max-reduced.
"""
import sys

if "/opt/trn_rl_repo" not in sys.path:
    sys.path.insert(0, "/opt/trn_rl_repo")

import numpy as np

EPS = 1e-5
_WS_CTR = [0]


def split_multiwait(nc, max_waits=1):
    """This walrus build rejects >1 sync-wait per instruction; hoist extras
    onto NoOp wait-carriers inserted before the instruction."""
    import concourse.mybir as mybir

    nsplit = 0
    for _, bbwrap in nc.bb_map.items():
        bb = bbwrap.bb
        newlist = []
        changed = False
        for inst in bb.instructions:
            si = inst.sync_info
            waits = list(si.on_wait) if si is not None and si.on_wait else []
            if len(waits) > max_waits:
                head = waits[:-max_waits]
                si.on_wait = waits[-max_waits:]
                for c0 in range(0, len(head), max_waits):
                    _WS_CTR[0] += 1
                    nop = mybir.InstNoOp(name=f"I-wsplit-{_WS_CTR[0]}", ins=[], outs=[])
                    nop.engine = inst.engine
                    nop.sync_info = mybir.SyncInfo(
                        on_wait=list(head[c0 : c0 + max_waits]), on_update=[]
                    )
                    nc.register_instruction(nop)
                    newlist.append(nop)
                changed = True
                nsplit += 1
            newlist.append(inst)
        if changed:
            bb.instructions = newlist
    return nsplit
B, N, K, NCLS, EMB = 4, 4096, 20, 50, 1024
NHALF = N // 2  # 2048
NT_OWN = NHALF // 128  # 16 query tiles per core
NT_ALL = N // 128  # 32
ECL = [
    # (name, C_in, O)
    ("ec1", 3, 64),
    ("ec2", 64, 64),
    ("ec3", 64, 128),
    ("ec4", 128, 256),
]


def _fold_bn(p):
    s = np.asarray(p["g"], np.float32) / np.sqrt(np.asarray(p["v"], np.float32) + EPS)
    t = np.asarray(p["b"], np.float32) - np.asarray(p["m"], np.float32) * s
    return s, t


def host_weights(params):
    """Folded weight tensors, shared across cores."""
    w = {}
    for name, C, O in ECL:
        p = params[name]
        pw = np.asarray(p["w"], np.float32)  # [O, 2C]
        s, t = _fold_bn(p)
        W1 = pw[:, :C]
        W2 = pw[:, C:]
        w[f"{name}_wu"] = np.ascontiguousarray((s[:, None] * W1).T)  # [C, O]
        w[f"{name}_wv"] = np.ascontiguousarray((s[:, None] * (W2 - W1)).T)  # [C, O]
        w[f"{name}_wvt"] = np.ascontiguousarray(t[None, :])  # [1, O]
    s, t = _fold_bn(params["fuse"])
    fw = (s[:, None] * np.asarray(params["fuse"]["w"], np.float32)).T  # [512, 1024]
    w["fuse_w"] = np.ascontiguousarray(fw)
    w["fuse_t"] = np.ascontiguousarray(t[None, :])  # [1, 1024]
    s, t = _fold_bn(params["h1"])
    h1w = s[:, None] * np.asarray(params["h1"]["w"], np.float32)  # [512, 1536]
    w["h1_w"] = np.ascontiguousarray(h1w[:, :512].T)  # [512 xcat, 512 ch]
    w["h1b_w"] = np.ascontiguousarray(
        np.concatenate([h1w[:, 512:].T, t[None, :]], axis=0)
    )  # [1025, 512]: bias' = h1b_w.T @ [g; 1]
    s, t = _fold_bn(params["h2"])
    w["h2_w"] = np.ascontiguousarray(
        (s[:, None] * np.asarray(params["h2"]["w"], np.float32)).T
    )  # [512, 256]
    w["h2_t"] = np.ascontiguousarray(t[None, :])  # [1, 256]
    w["h3_w"] = np.ascontiguousarray(np.asarray(params["h3"]["w"], np.float32).T)  # [256, 50]
    w["h3_t"] = np.ascontiguousarray(np.asarray(params["h3"]["bias"], np.float32)[None, :])
    w["ident"] = np.eye(128, dtype=np.float32)
    return w


def host_core_inputs(xyz, c):
    """Per-core layer-1 feature tensors in core-relative (own-half-first) order.

    The distance score is s = x_q . x_c - xx_c/2 (the query-side xx term is a
    per-row constant and cannot change that row's top-k)."""
    h = c % 2
    x = np.asarray(xyz[c // 2], np.float32)  # [4096, 3]
    perm = np.concatenate(
        [
            np.arange(h * NHALF, (h + 1) * NHALF),
            np.arange((1 - h) * NHALF, (2 - h) * NHALF) % N,
        ]
    )
    xp = x[perm]
    nxx = -0.5 * (xp * xp).sum(-1)
    return {
        "xT": np.ascontiguousarray(xp.T),
        "nxx": np.ascontiguousarray(nxx[None, :]),
    }


_PROG_CACHE = {}


def build_program():
    import contextlib

    import concourse.bass as bass
    import concourse.mybir as mybir
    from concourse.bass import IndirectOffsetOnAxis
    from concourse.tile import TileContext

    import os
    F32 = mybir.dt.float32
    F32R = mybir.dt.float32r
    DIST_R = bool(os.environ.get("KDIST_F32R"))
    MLP_R = bool(os.environ.get("KMLP_F32R"))
    def dr(ap):
        return ap.bitcast(F32R) if DIST_R else ap
    def mr_(ap):
        return ap.bitcast(F32R) if MLP_R else ap
    U16 = mybir.dt.uint16
    U32 = mybir.dt.uint32
    AF = mybir.ActivationFunctionType
    ALU = mybir.AluOpType
    AX = mybir.AxisListType
    RG = [[0, 1], [2, 3], [4, 5], [6, 7]]

    nc = bass.Bass("TRN2", target_bir_lowering=False, debug=False)

    xT_d = nc.declare_dram_parameter("xT", [3, N], F32, isOutput=False)
    nxx_d = nc.declare_dram_parameter("nxx", [1, N], F32, isOutput=False)
    wd = {}
    for name, C, O in ECL:
        wd[f"{name}_wu"] = nc.declare_dram_parameter(f"{name}_wu", [C, O], F32, isOutput=False)
        wd[f"{name}_wv"] = nc.declare_dram_parameter(f"{name}_wv", [C, O], F32, isOutput=False)
        wd[f"{name}_wvt"] = nc.declare_dram_parameter(f"{name}_wvt", [1, O], F32, isOutput=False)
    wd["fuse_w"] = nc.declare_dram_parameter("fuse_w", [512, EMB], F32, isOutput=False)
    wd["fuse_t"] = nc.declare_dram_parameter("fuse_t", [1, EMB], F32, isOutput=False)
    wd["h1_w"] = nc.declare_dram_parameter("h1_w", [512, 512], F32, isOutput=False)
    wd["h1b_w"] = nc.declare_dram_parameter("h1b_w", [EMB + 1, 512], F32, isOutput=False)
    wd["h2_w"] = nc.declare_dram_parameter("h2_w", [512, 256], F32, isOutput=False)
    wd["h2_t"] = nc.declare_dram_parameter("h2_t", [1, 256], F32, isOutput=False)
    wd["h3_w"] = nc.declare_dram_parameter("h3_w", [256, NCLS], F32, isOutput=False)
    wd["h3_t"] = nc.declare_dram_parameter("h3_t", [1, NCLS], F32, isOutput=False)
    wd["ident"] = nc.declare_dram_parameter("ident", [128, 128], F32, isOutput=False)
    out_d = nc.declare_dram_parameter("out", [NHALF, NCLS], F32, isOutput=True)

    import os as _os
    _pam = "stack" if _os.environ.get("KSTACK") else "queue"
    with TileContext(nc, num_cores=8, pool_alloc_mode=_pam) as tc, contextlib.ExitStack() as stack:
        persist = stack.enter_context(tc.tile_pool(name="persist", bufs=1))
        dramp = stack.enter_context(tc.tile_pool(name="dramp", bufs=1, space="DRAM"))
        u_dram = {
            name: dramp.tile([N, O], F32, tag=f"u_{name}", name=f"u_{name}")
            for name, C, O in ECL
        }
        cc_in = {}
        cc_out = {}
        for li, (name, C, O) in enumerate(ECL[:3]):
            cc_in[name] = dramp.tile([O, NHALF], F32, tag=f"cc_in_{name}", name=f"cc_in_{name}")
            cc_out[name] = dramp.tile([O, NHALF], F32, tag=f"cc_out_{name}", name=f"cc_out_{name}")
        g_in = dramp.tile([128, 8], F32, tag="g_in", name="g_in")
        g_out = dramp.tile([128, 8], F32, tag="g_out", name="g_out")

        ones512 = persist.tile([1, 512], F32, tag="ones512")
        nc.vector.memset(ones512[:], 1.0)
        one_cell = persist.tile([1, 1], F32, tag="one_cell")
        nc.vector.memset(one_cell[:], 1.0)
        ident_sb = persist.tile([128, 128], F32, tag="ident")
        nc.sync.dma_start(ident_sb[:], wd["ident"][:, :])

        # transposed feature tensors; y1..y3 full cloud, y4 own half only
        yT = {
            "ec1": [persist.tile([64, N], F32, tag="y1T", name="y1T")],
            "ec2": [persist.tile([64, N], F32, tag="y2T", name="y2T")],
            "ec3": [persist.tile([128, N], F32, tag="y3T", name="y3T")],
            "ec4": [
                persist.tile([128, NHALF], F32, tag="y4Ta", name="y4Ta"),
                persist.tile([128, NHALF], F32, tag="y4Tb", name="y4Tb"),
            ],
        }
        xm1 = persist.tile([3, N], F32, tag="xm1")
        nc.sync.dma_start(xm1[:], xT_d[:, :])

        # -xx/2 row of the candidate cloud; layer 1 from host, l>=2 on device
        nxx_row = persist.tile([1, N], F32, tag="nxx_row")
        nc.sync.dma_start(nxx_row[:], nxx_d[:, :])

        ecw = {}
        for name, C, O in ECL:
            for p in ("wu", "wv", "wvt"):
                shp = [1, O] if p == "wvt" else [C, O]
                tl = persist.tile(shp, F32, tag=f"{name}_{p}", name=f"{name}_{p}_sb")
                nc.sync.dma_start(tl[:], wd[f"{name}_{p}"][:, :])
                ecw[f"{name}_{p}"] = tl

        def feat_chunks(li):
            if li == 0:
                return [(xm1, 3)]
            return [(t, t.shape[0]) for t in yT[ECL[li - 1][0]]]

        # ================= edge conv layers =================
        for li, (name, C, O) in enumerate(ECL):
            if li > 0:
                with (
                    tc.tile_pool(name=f"xx_{name}", bufs=2) as xxp,
                    tc.tile_pool(name=f"xxps_{name}", bufs=2, space="PSUM") as xxps,
                ):
                    onescol = xxp.tile([128, 1], F32, tag="onescol")
                    nc.vector.memset(onescol[:], 1.0)
                    fcs = feat_chunks(li)
                    for cb in range(8):
                        ps = xxps.tile([1, 512], F32)
                        for ci, (t, nr) in enumerate(fcs):
                            sq = xxp.tile([128, 512], F32, tag="sq")
                            nc.scalar.activation(
                                sq[:nr, :], t[:nr, cb * 512 : (cb + 1) * 512], AF.Square
                            )
                            nc.tensor.matmul(
                                ps[:],
                                onescol[:nr, :],
                                sq[:nr, :],
                                start=(ci == 0),
                                stop=(ci == len(fcs) - 1),
                            )
                        sl = slice(cb * 512, (cb + 1) * 512)
                        nc.scalar.activation(nxx_row[0:1, sl], ps[:], AF.Copy, scale=-0.5)

            # ---- u for the full cloud -> u_dram ----
            with (
                tc.tile_pool(name=f"u_{name}", bufs=3) as up,
                tc.tile_pool(name=f"ups_{name}", bufs=2, space="PSUM") as ups,
            ):
                fcs = feat_chunks(li)
                for jt in range(NT_ALL):
                    ps = ups.tile([128, O], F32)
                    r0 = 0
                    for ci, (t, nr) in enumerate(fcs):
                        nc.tensor.matmul(
                            ps[:],
                            t[:nr, jt * 128 : (jt + 1) * 128],
                            ecw[f"{name}_wu"][r0 : r0 + nr, :],
                            start=(ci == 0),
                            stop=(ci == len(fcs) - 1),
                        )
                        r0 += nr
                    us = up.tile([128, O], F32, tag="us")
                    nc.scalar.activation(us[:], ps[:], AF.Copy)
                    nc.sync.dma_start(u_dram[name][jt * 128 : (jt + 1) * 128, :], us[:])

            with tc.tile_pool(name=f"lay_{name}", bufs=1) as lay:
                # ---- v' own half ----
                v_sb = lay.tile([128, NT_OWN * O], F32, tag="v_sb")
                with tc.tile_pool(name=f"vps_{name}", bufs=2, space="PSUM") as vps:
                    fcs = feat_chunks(li)
                    for qt in range(NT_OWN):
                        ps = vps.tile([128, O], F32)
                        r0 = 0
                        for t, nr in fcs:
                            nc.tensor.matmul(
                                ps[:],
                                t[:nr, qt * 128 : (qt + 1) * 128],
                                ecw[f"{name}_wv"][r0 : r0 + nr, :],
                                start=(r0 == 0),
                                stop=False,
                            )
                            r0 += nr
                        nc.tensor.matmul(
                            ps[:],
                            ones512[:, 0:128],
                            ecw[f"{name}_wvt"][:, :],
                            start=False,
                            stop=True,
                        )
                        nc.scalar.activation(v_sb[:, qt * O : (qt + 1) * O], ps[:], AF.Copy)

                # ---- distances + topk + gather + neighbour max ----
                ynat = lay.tile([128, NT_OWN * O], F32, tag="ynat")
                with (
                    tc.tile_pool(name=f"s_{name}", bufs=3, space="PSUM") as sps,
                    tc.tile_pool(name=f"tr_{name}", bufs=2, space="PSUM") as trp,
                    tc.tile_pool(name=f"tk_{name}", bufs=3) as tkp,
                    tc.tile_pool(name=f"ga_{name}", bufs=(1 if O == 256 else 3)) as gap,
                ):
                    fcs = feat_chunks(li)
                    for qt in range(NT_OWN):
                        # per-bank top-16 (values + bank-local indices), read
                        # straight from PSUM; round-0 match_replace evicts the
                        # bank row to SBUF for round 1.
                        bvals = tkp.tile([128, 8, 16], F32, tag="bvals")
                        bidx = tkp.tile([128, 8, 16], U16, tag="bidx")
                        for hb in range(4):  # 2-bank PSUM chunks
                            s_ps = sps.tile([128, N // 4], F32, tag="s_ps")
                            for cb in range(2):
                                for ci, (st, snr) in enumerate(fcs):
                                    nc.tensor.matmul(
                                        s_ps[:, cb * 512 : (cb + 1) * 512],
                                        dr(st[:snr, qt * 128 : (qt + 1) * 128]),
                                        dr(st[:snr, (hb * 2 + cb) * 512 : (hb * 2 + cb + 1) * 512]),
                                        start=(ci == 0),
                                        stop=False,
                                    )
                                nc.tensor.matmul(
                                    s_ps[:, cb * 512 : (cb + 1) * 512],
                                    dr(ones512[:, 0:128]),
                                    dr(nxx_row[:, (hb * 2 + cb) * 512 : (hb * 2 + cb + 1) * 512]),
                                    start=False,
                                    stop=True,
                                )
                            for cb in range(2):
                                b = hb * 2 + cb
                                row = tkp.tile([128, 512], F32, tag="row_sb")
                                nc.scalar.activation(
                                    row[:], s_ps[:, cb * 512 : (cb + 1) * 512], AF.Copy
                                )
                                v8 = bvals[:, b, 0:8]
                                i8 = bidx[:, b, 0:8]
                                nc.vector.max(out=v8, in_=row[:])
                                nc.vector.max_index(out=i8, in_max=v8, in_values=row[:])
                                nc.vector.match_replace(
                                    out=row[:], in_to_replace=v8, in_values=row[:],
                                    imm_value=-1e30,
                                )
                                v8b = bvals[:, b, 8:16]
                                i8b = bidx[:, b, 8:16]
                                nc.vector.max(out=v8b, in_=row[:])
                                nc.vector.max_index(out=i8b, in_max=v8b, in_values=row[:])
                        # global candidate arrays [128, 128]
                        idxf = tkp.tile([128, 8, 16], F32, tag="idxf")
                        nc.vector.tensor_copy(idxf[:], bidx[:])
                        for cb in range(8):
                            if cb:
                                nc.vector.tensor_scalar_add(
                                    idxf[:, cb, :], idxf[:, cb, :], float(cb * 512)
                                )
                        # top-24 values of the 128 candidates -> t20
                        mv = tkp.tile([128, 24], F32, tag="mv")
                        vwork = tkp.tile([128, 128], F32, tag="vwork")
                        bv_flat = bvals[:].rearrange("p a b -> p (a b)")
                        v8 = mv[:, 0:8]
                        nc.vector.max(out=v8, in_=bv_flat)
                        nc.vector.match_replace(
                            out=vwork[:], in_to_replace=v8, in_values=bv_flat,
                            imm_value=-1e30,
                        )
                        for r in (1, 2):
                            v8 = mv[:, r * 8 : (r + 1) * 8]
                            nc.vector.max(out=v8, in_=vwork[:])
                            if r < 2:
                                nc.vector.match_replace(
                                    out=vwork[:], in_to_replace=v8, in_values=vwork[:],
                                    imm_value=-1e30,
                                )
                        # seli = idxf + (bvals < t20) * -1e30   (t20 = mv[:, 19])
                        seli = tkp.tile([128, 128], F32, tag="seli")
                        nc.vector.tensor_scalar(
                            seli[:],
                            bv_flat,
                            mv[:, 19:20],
                            -1e30,
                            op0=ALU.is_lt,
                            op1=ALU.mult,
                        )
                        nc.vector.tensor_tensor(
                            out=seli[:],
                            in0=seli[:],
                            in1=idxf[:].rearrange("p a b -> p (a b)"),
                            op=ALU.add,
                        )
                        sidx = tkp.tile([128, 24], F32, tag="sidx")
                        for r in range(3):
                            v8 = sidx[:, r * 8 : (r + 1) * 8]
                            nc.vector.max(out=v8, in_=seli[:])
                            if r < 2:
                                nc.vector.match_replace(
                                    out=seli[:], in_to_replace=v8, in_values=seli[:],
                                    imm_value=-1e30,
                                )
                        idx32 = tkp.tile([128, K], U32, tag="idx32")
                        nc.vector.tensor_copy(idx32[:], sidx[:, 0:K])
                        gath = gap.tile([128, K, O], F32, tag="gath")
                        for j in range(K):
                            nc.gpsimd.indirect_dma_start(
                                out=gath[:, j, :],
                                out_offset=None,
                                in_=u_dram[name][:, :],
                                in_offset=IndirectOffsetOnAxis(
                                    ap=idx32[:, j : j + 1], axis=0
                                ),
                            )
                        m_sb = tkp.tile([128, O], F32, tag="m_sb")
                        nc.vector.tensor_reduce(
                            out=m_sb[:],
                            in_=gath[:].rearrange("p k o -> p o k"),
                            axis=AX.X,
                            op=ALU.max,
                        )
                        nc.vector.tensor_add(
                            out=m_sb[:], in0=m_sb[:], in1=v_sb[:, qt * O : (qt + 1) * O]
                        )
                        nc.scalar.activation(
                            ynat[:, qt * O : (qt + 1) * O], m_sb[:], AF.Prelu, alpha=0.2
                        )
                        # transpose this tile's own-half columns immediately so
                        # the transpose work overlaps later tiles' extraction
                        for oc in range(0, O, 128):
                            ow = min(128, O - oc)
                            tp = trp.tile([128, 128], F32)
                            nc.tensor.transpose(
                                tp[:ow, :],
                                ynat[:, qt * O + oc : qt * O + oc + ow],
                                ident_sb[:],
                            )
                            dst = yT[name][oc // 128]
                            dslice = dst[:ow, qt * 128 : (qt + 1) * 128]
                            if li == 3:
                                # y4 feeds only the MLP: store fp32r-rounded so
                                # the fuse/h1 matmuls can run in fp32r mode.
                                dslice = dslice.bitcast(F32R)
                            nc.scalar.activation(dslice, tp[:ow, :], AF.Copy)

            # ---- exchange halves (layers 1-3): partner = allreduce_sum - own ----
            if li < 3:
                ytl = yT[name]
                for ci_, t in enumerate(ytl):
                    nr = t.shape[0]
                    nc.sync.dma_start(
                        cc_in[name][ci_ * 128 : ci_ * 128 + nr, :], t[:nr, 0:NHALF]
                    )
                nc.gpsimd.collective_compute(
                    "AllReduce",
                    ALU.add,
                    replica_groups=RG,
                    ins=[cc_in[name][:]],
                    outs=[cc_out[name][:]],
                )
                with tc.tile_pool(name=f"ex_{name}", bufs=2) as exp_:
                    for ci_, t in enumerate(ytl):
                        nr = t.shape[0]
                        ssum = exp_.tile([128, NHALF], F32, tag="ssum")
                        nc.sync.dma_start(
                            ssum[:nr, :], cc_out[name][ci_ * 128 : ci_ * 128 + nr, :]
                        )
                        nc.vector.tensor_tensor(
                            out=t[:nr, NHALF:N],
                            in0=ssum[:nr, :],
                            in1=t[:nr, 0:NHALF],
                            op=ALU.subtract,
                        )

        # ================= MLP head (own half) =================
        xcat_chunks = [
            (yT["ec1"][0], 64),
            (yT["ec2"][0], 64),
            (yT["ec3"][0], 128),
            (yT["ec4"][0], 128),
            (yT["ec4"][1], 128),
        ]
        mlp = stack.enter_context(tc.tile_pool(name="mlp", bufs=1))

        g_sb = mlp.tile([128, 8], F32, tag="g_sb")
        with (
            tc.tile_pool(name="fw", bufs=2) as fwp,
            tc.tile_pool(name="fps", bufs=2, space="PSUM") as fps,
            tc.tile_pool(name="fsc", bufs=2) as fsc,
        ):
            for ec in range(8):
                fw_sb = []
                r0 = 0
                for wci, (t, nr) in enumerate(xcat_chunks):
                    wsb = fwp.tile([128, 128], F32, tag=f"fw_sb{wci}", name=f"fw_sb{wci}")
                    nc.sync.dma_start(
                        wsb[:nr, :], wd["fuse_w"][r0 : r0 + nr, ec * 128 : (ec + 1) * 128]
                    )
                    if wci >= 3:  # y4 chunks run in fp32r
                        wr = fwp.tile([128, 128], F32R, tag=f"fw_r{wci}", name=f"fw_r{wci}")
                        nc.vector.tensor_copy(wr[:nr, :], wsb[:nr, :])
                        wsb = wr
                    fw_sb.append((wsb, nr))
                    r0 += nr
                ft_sb = fwp.tile([1, 128], F32, tag="ft_sb")
                nc.sync.dma_start(ft_sb[:], wd["fuse_t"][:, ec * 128 : (ec + 1) * 128])
                for fb in range(4):
                    ps = fps.tile([128, 512], F32)
                    for ci, ((t, nr), (wsb, _)) in enumerate(zip(xcat_chunks, fw_sb)):
                        tt = t[:nr, fb * 512 : (fb + 1) * 512]
                        if ci >= 3:
                            tt = tt.bitcast(F32R)
                        nc.tensor.matmul(
                            ps[:], wsb[:nr, :], tt, start=(ci == 0), stop=False
                        )
                    nc.tensor.matmul(
                        ps[:], ft_sb[:, :], ones512[:, :], start=False, stop=True
                    )
                    fch = fsc.tile([128, 512], F32, tag="fch")
                    nc.scalar.activation(fch[:], ps[:], AF.Prelu, alpha=0.2)
                    red = fsc.tile([128, 1], F32, tag="red")
                    nc.vector.tensor_reduce(out=red[:], in_=fch[:], axis=AX.X, op=ALU.max)
                    if fb == 0:
                        nc.vector.tensor_copy(g_sb[:, ec : ec + 1], red[:])
                    else:
                        nc.vector.tensor_tensor(
                            out=g_sb[:, ec : ec + 1],
                            in0=g_sb[:, ec : ec + 1],
                            in1=red[:],
                            op=ALU.max,
                        )

        nc.sync.dma_start(g_in[:, :], g_sb[:])
        nc.gpsimd.collective_compute(
            "AllReduce", ALU.max, replica_groups=RG, ins=[g_in[:]], outs=[g_out[:]]
        )
        g2_sb = mlp.tile([128, 8], F32, tag="g2_sb")
        nc.sync.dma_start(g2_sb[:], g_out[:, :])

        bias_sb = mlp.tile([1, 512], F32, tag="bias_sb")
        with (
            tc.tile_pool(name="bw", bufs=2) as bwp,
            tc.tile_pool(name="bps", bufs=1, space="PSUM") as bps,
        ):
            ps = bps.tile([1, 512], F32)
            for kc in range(8):
                wsb = bwp.tile([128, 512], F32, tag="bw_sb")
                nc.sync.dma_start(wsb[:], wd["h1b_w"][kc * 128 : (kc + 1) * 128, :])
                nc.tensor.matmul(
                    ps[:], g2_sb[:, kc : kc + 1], wsb[:], start=(kc == 0), stop=False
                )
            wtb = bwp.tile([1, 512], F32, tag="bt_sb")
            nc.sync.dma_start(wtb[:], wd["h1b_w"][EMB : EMB + 1, :])
            nc.tensor.matmul(ps[:], one_cell[:], wtb[:], start=False, stop=True)
            nc.scalar.activation(bias_sb[:], ps[:], AF.Copy)

        h1T = [mlp.tile([128, NHALF], F32, tag=f"h1T_{i}", name=f"h1T_{i}") for i in range(4)]
        with (
            tc.tile_pool(name="h1w", bufs=2) as h1wp,
            tc.tile_pool(name="h1ps", bufs=2, space="PSUM") as h1ps,
        ):
            for cc in range(4):
                w_sb = []
                r0 = 0
                for wci, (t, nr) in enumerate(xcat_chunks):
                    wsb = h1wp.tile([128, 128], F32, tag=f"h1w_sb{wci}", name=f"h1w_sb{wci}")
                    nc.sync.dma_start(
                        wsb[:nr, :], wd["h1_w"][r0 : r0 + nr, cc * 128 : (cc + 1) * 128]
                    )
                    if wci >= 3:
                        wr = h1wp.tile([128, 128], F32R, tag=f"h1w_r{wci}", name=f"h1w_r{wci}")
                        nc.vector.tensor_copy(wr[:nr, :], wsb[:nr, :])
                        wsb = wr
                    w_sb.append((wsb, nr))
                    r0 += nr
                for fb in range(4):
                    ps = h1ps.tile([128, 512], F32)
                    for ci, ((t, nr), (wsb, _)) in enumerate(zip(xcat_chunks, w_sb)):
                        tt = t[:nr, fb * 512 : (fb + 1) * 512]
                        if ci >= 3:
                            tt = tt.bitcast(F32R)
                        nc.tensor.matmul(
                            ps[:], wsb[:nr, :], tt, start=(ci == 0), stop=False
                        )
                    nc.tensor.matmul(
                        ps[:],
                        bias_sb[:, cc * 128 : (cc + 1) * 128],
                        ones512[:, :],
                        start=False,
                        stop=True,
                    )
                    nc.scalar.activation(
                        h1T[cc][:, fb * 512 : (fb + 1) * 512].bitcast(F32R),
                        ps[:],
                        AF.Prelu,
                        alpha=0.2,
                    )

        h2T = [mlp.tile([128, NHALF], F32, tag=f"h2T_{i}", name=f"h2T_{i}") for i in range(2)]
        with (
            tc.tile_pool(name="h2w", bufs=2) as h2wp,
            tc.tile_pool(name="h2ps", bufs=2, space="PSUM") as h2ps,
        ):
            for cc in range(2):
                w_sb = []
                for kc in range(4):
                    wst = h2wp.tile([128, 128], F32, tag=f"h2w_st{kc}", name=f"h2w_st{kc}")
                    nc.sync.dma_start(
                        wst[:],
                        wd["h2_w"][kc * 128 : (kc + 1) * 128, cc * 128 : (cc + 1) * 128],
                    )
                    wsb = h2wp.tile([128, 128], F32R, tag=f"h2w_sb{kc}", name=f"h2w_sb{kc}")
                    nc.vector.tensor_copy(wsb[:], wst[:])
                    w_sb.append(wsb)
                t_sb = h2wp.tile([1, 128], F32, tag="h2t_sb")
                nc.sync.dma_start(t_sb[:], wd["h2_t"][:, cc * 128 : (cc + 1) * 128])
                for fb in range(4):
                    ps = h2ps.tile([128, 512], F32)
                    for kc in range(4):
                        nc.tensor.matmul(
                            ps[:],
                            w_sb[kc][:, :],
                            h1T[kc][:, fb * 512 : (fb + 1) * 512].bitcast(F32R),
                            start=(kc == 0),
                            stop=False,
                        )
                    nc.tensor.matmul(
                        ps[:], t_sb[:, :], ones512[:, :], start=False, stop=True
                    )
                    nc.scalar.activation(
                        h2T[cc][:, fb * 512 : (fb + 1) * 512], ps[:], AF.Prelu, alpha=0.2
                    )

        with (
            tc.tile_pool(name="h3w", bufs=1) as h3wp,
            tc.tile_pool(name="h3ps", bufs=2, space="PSUM") as h3ps,
            tc.tile_pool(name="h3o", bufs=3) as h3op,
        ):
            w_sb = h3wp.tile([128, 2, NCLS], F32, tag="h3w_sb")
            nc.sync.dma_start(w_sb[:, 0, :], wd["h3_w"][0:128, :])
            nc.sync.dma_start(w_sb[:, 1, :], wd["h3_w"][128:256, :])
            t_sb = h3wp.tile([1, NCLS], F32, tag="h3t_sb")
            nc.sync.dma_start(t_sb[:], wd["h3_t"][:, :])
            for qt in range(NT_OWN):
                ps = h3ps.tile([128, NCLS], F32)
                for kc in range(2):
                    nc.tensor.matmul(
                        ps[:],
                        h2T[kc][:, qt * 128 : (qt + 1) * 128],
                        w_sb[:, kc, :],
                        start=(kc == 0),
                        stop=False,
                    )
                nc.tensor.matmul(
                    ps[:], ones512[:, 0:128], t_sb[:, :], start=False, stop=True
                )
                osb = h3op.tile([128, NCLS], F32, tag="osb")
                nc.scalar.activation(osb[:], ps[:], AF.Copy)
                nc.sync.dma_start(out_d[qt * 128 : (qt + 1) * 128, :], osb[:])

    split_multiwait(nc)
    return nc


def kernel(xyz, params, trace=False):
    import os

    import concourse.bass_utils as bass_utils

    xyz = np.asarray(xyz, np.float32)
    if "prog" not in _PROG_CACHE:
        _PROG_CACHE["prog"] = build_program()
    nc = _PROG_CACHE["prog"]

    wshared = host_weights(params)
    core_ids = list(range(8))
    in_maps = []
    for c in core_ids:
        m = dict(wshared)
        m.update(host_core_inputs(xyz, c))
        in_maps.append(m)
    res = bass_utils.run_bass_kernel_spmd(
        nc, in_maps, core_ids, trace=trace or bool(os.environ.get("KTRACE"))
    )
    _PROG_CACHE["last_res"] = res
    out = np.zeros((B, N, NCLS), np.float32)
    for c in core_ids:
        b, h = c // 2, c % 2
        out[b, h * NHALF : (h + 1) * NHALF, :] = res.results[c]["out"]
    return out


# revision 35
# speedup vs baseline: 1.1007x; 1.1007x over previous
"""DGCNN segmentation Bass kernel for 8 trn2 NeuronCores.

Sharding: data-parallel over (cloud b, half h): core c = 2*b + h owns
queries [h*2048:(h+1)*2048] of cloud b. Candidates are kept in a
core-relative order (own half first) so the SPMD program is identical on
every core; halves are exchanged per layer with a pairwise AllReduce(add)
(partner = sum - own).

EdgeConv is linearized: W @ [xj - xi; xi] = W1 @ xj + (W2 - W1) @ xi, and
eval-mode BN is folded into the weights on the host. LeakyReLU(0.2) is the
ACT engine's Prelu. Per-query top-20 neighbours come from 3 rounds of
max8/max_index/match_replace on the negative half-distance row; neighbour
feature rows are gathered with per-partition indirect DMA and max-reduced.
"""
import sys

if "/opt/trn_rl_repo" not in sys.path:
    sys.path.insert(0, "/opt/trn_rl_repo")

import numpy as np

EPS = 1e-5
_WS_CTR = [0]


def split_multiwait(nc, max_waits=1):
    """This walrus build rejects >1 sync-wait per instruction; hoist extras
    onto NoOp wait-carriers inserted before the instruction."""
    import concourse.mybir as mybir

    nsplit = 0
    for _, bbwrap in nc.bb_map.items():
        bb = bbwrap.bb
        newlist = []
        changed = False
        for inst in bb.instructions:
            si = inst.sync_info
            waits = list(si.on_wait) if si is not None and si.on_wait else []
            if len(waits) > max_waits:
                head = waits[:-max_waits]
                si.on_wait = waits[-max_waits:]
                for c0 in range(0, len(head), max_waits):
                    _WS_CTR[0] += 1
                    nop = mybir.InstNoOp(name=f"I-wsplit-{_WS_CTR[0]}", ins=[], outs=[])
                    nop.engine = inst.engine
                    nop.sync_info = mybir.SyncInfo(
                        on_wait=list(head[c0 : c0 + max_waits]), on_update=[]
                    )
                    nc.register_instruction(nop)
                    newlist.append(nop)
                changed = True
                nsplit += 1
            newlist.append(inst)
        if changed:
            bb.instructions = newlist
    return nsplit
B, N, K, NCLS, EMB = 4, 4096, 20, 50, 1024
NHALF = N // 2  # 2048
NT_OWN = NHALF // 128  # 16 query tiles per core
NT_ALL = N // 128  # 32
ECL = [
    # (name, C_in, O)
    ("ec1", 3, 64),
    ("ec2", 64, 64),
    ("ec3", 64, 128),
    ("ec4", 128, 256),
]


def _fold_bn(p):
    s = np.asarray(p["g"], np.float32) / np.sqrt(np.asarray(p["v"], np.float32) + EPS)
    t = np.asarray(p["b"], np.float32) - np.asarray(p["m"], np.float32) * s
    return s, t


def host_weights(params):
    """Folded weight tensors, shared across cores."""
    w = {}
    for name, C, O in ECL:
        p = params[name]
        pw = np.asarray(p["w"], np.float32)  # [O, 2C]
        s, t = _fold_bn(p)
        W1 = pw[:, :C]
        W2 = pw[:, C:]
        w[f"{name}_wu"] = np.ascontiguousarray((s[:, None] * W1).T)  # [C, O]
        w[f"{name}_wv"] = np.ascontiguousarray((s[:, None] * (W2 - W1)).T)  # [C, O]
        w[f"{name}_wvt"] = np.ascontiguousarray(t[None, :])  # [1, O]
    s, t = _fold_bn(params["fuse"])
    fw = (s[:, None] * np.asarray(params["fuse"]["w"], np.float32)).T  # [512, 1024]
    w["fuse_w"] = np.ascontiguousarray(fw)
    w["fuse_t"] = np.ascontiguousarray(t[None, :])  # [1, 1024]
    s, t = _fold_bn(params["h1"])
    h1w = s[:, None] * np.asarray(params["h1"]["w"], np.float32)  # [512, 1536]
    w["h1_w"] = np.ascontiguousarray(h1w[:, :512].T)  # [512 xcat, 512 ch]
    w["h1b_w"] = np.ascontiguousarray(
        np.concatenate([h1w[:, 512:].T, t[None, :]], axis=0)
    )  # [1025, 512]: bias' = h1b_w.T @ [g; 1]
    s, t = _fold_bn(params["h2"])
    w["h2_w"] = np.ascontiguousarray(
        (s[:, None] * np.asarray(params["h2"]["w"], np.float32)).T
    )  # [512, 256]
    w["h2_t"] = np.ascontiguousarray(t[None, :])  # [1, 256]
    w["h3_w"] = np.ascontiguousarray(np.asarray(params["h3"]["w"], np.float32).T)  # [256, 50]
    w["h3_t"] = np.ascontiguousarray(np.asarray(params["h3"]["bias"], np.float32)[None, :])
    w["ident"] = np.eye(128, dtype=np.float32)
    return w


def host_core_inputs(xyz, c):
    """Per-core layer-1 feature tensors in core-relative (own-half-first) order.

    The distance score is s = x_q . x_c - xx_c/2 (the query-side xx term is a
    per-row constant and cannot change that row's top-k)."""
    h = c % 2
    x = np.asarray(xyz[c // 2], np.float32)  # [4096, 3]
    perm = np.concatenate(
        [
            np.arange(h * NHALF, (h + 1) * NHALF),
            np.arange((1 - h) * NHALF, (2 - h) * NHALF) % N,
        ]
    )
    xp = x[perm]
    nxx = -0.5 * (xp * xp).sum(-1)
    return {
        "xT": np.ascontiguousarray(xp.T),
        "nxx": np.ascontiguousarray(nxx[None, :]),
    }


_PROG_CACHE = {}


def build_program():
    import contextlib

    import concourse.bass as bass
    import concourse.mybir as mybir
    from concourse.bass import IndirectOffsetOnAxis
    from concourse.tile import TileContext

    import os
    F32 = mybir.dt.float32
    F32R = mybir.dt.float32r
    DIST_R = bool(os.environ.get("KDIST_F32R"))
    MLP_R = bool(os.environ.get("KMLP_F32R"))
    def dr(ap):
        return ap.bitcast(F32R) if DIST_R else ap
    def mr_(ap):
        return ap.bitcast(F32R) if MLP_R else ap
    U16 = mybir.dt.uint16
    U32 = mybir.dt.uint32
    AF = mybir.ActivationFunctionType
    ALU = mybir.AluOpType
    AX = mybir.AxisListType
    RG = [[0, 1], [2, 3], [4, 5], [6, 7]]

    nc = bass.Bass("TRN2", target_bir_lowering=False, debug=False)

    xT_d = nc.declare_dram_parameter("xT", [3, N], F32, isOutput=False)
    nxx_d = nc.declare_dram_parameter("nxx", [1, N], F32, isOutput=False)
    wd = {}
    for name, C, O in ECL:
        wd[f"{name}_wu"] = nc.declare_dram_parameter(f"{name}_wu", [C, O], F32, isOutput=False)
        wd[f"{name}_wv"] = nc.declare_dram_parameter(f"{name}_wv", [C, O], F32, isOutput=False)
        wd[f"{name}_wvt"] = nc.declare_dram_parameter(f"{name}_wvt", [1, O], F32, isOutput=False)
    wd["fuse_w"] = nc.declare_dram_parameter("fuse_w", [512, EMB], F32, isOutput=False)
    wd["fuse_t"] = nc.declare_dram_parameter("fuse_t", [1, EMB], F32, isOutput=False)
    wd["h1_w"] = nc.declare_dram_parameter("h1_w", [512, 512], F32, isOutput=False)
    wd["h1b_w"] = nc.declare_dram_parameter("h1b_w", [EMB + 1, 512], F32, isOutput=False)
    wd["h2_w"] = nc.declare_dram_parameter("h2_w", [512, 256], F32, isOutput=False)
    wd["h2_t"] = nc.declare_dram_parameter("h2_t", [1, 256], F32, isOutput=False)
    wd["h3_w"] = nc.declare_dram_parameter("h3_w", [256, NCLS], F32, isOutput=False)
    wd["h3_t"] = nc.declare_dram_parameter("h3_t", [1, NCLS], F32, isOutput=False)
    wd["ident"] = nc.declare_dram_parameter("ident", [128, 128], F32, isOutput=False)
    out_d = nc.declare_dram_parameter("out", [NHALF, NCLS], F32, isOutput=True)

    import os as _os
    _pam = "stack" if _os.environ.get("KSTACK") else "queue"
    with TileContext(nc, num_cores=8, pool_alloc_mode=_pam) as tc, contextlib.ExitStack() as stack:
        persist = stack.enter_context(tc.tile_pool(name="persist", bufs=1))
        dramp = stack.enter_context(tc.tile_pool(name="dramp", bufs=1, space="DRAM"))
        u_dram = {
            name: dramp.tile([N, O], F32, tag=f"u_{name}", name=f"u_{name}")
            for name, C, O in ECL
        }
        cc_in = {}
        cc_out = {}
        for li, (name, C, O) in enumerate(ECL[:3]):
            cc_in[name] = dramp.tile([O, NHALF], F32, tag=f"cc_in_{name}", name=f"cc_in_{name}")
            cc_out[name] = dramp.tile([O, NHALF], F32, tag=f"cc_out_{name}", name=f"cc_out_{name}")
        g_in = dramp.tile([128, 8], F32, tag="g_in", name="g_in")
        g_out = dramp.tile([128, 8], F32, tag="g_out", name="g_out")

        ones512 = persist.tile([1, 512], F32, tag="ones512")
        nc.vector.memset(ones512[:], 1.0)
        one_cell = persist.tile([1, 1], F32, tag="one_cell")
        nc.vector.memset(one_cell[:], 1.0)
        ident_sb = persist.tile([128, 128], F32, tag="ident")
        nc.sync.dma_start(ident_sb[:], wd["ident"][:, :])

        # transposed feature tensors; y1..y3 full cloud, y4 own half only
        yT = {
            "ec1": [persist.tile([64, N], F32, tag="y1T", name="y1T")],
            "ec2": [persist.tile([64, N], F32, tag="y2T", name="y2T")],
            "ec3": [persist.tile([128, N], F32, tag="y3T", name="y3T")],
            "ec4": [
                persist.tile([128, NHALF], F32, tag="y4Ta", name="y4Ta"),
                persist.tile([128, NHALF], F32, tag="y4Tb", name="y4Tb"),
            ],
        }
        xm1 = persist.tile([3, N], F32, tag="xm1")
        nc.sync.dma_start(xm1[:], xT_d[:, :])

        # -xx/2 row of the candidate cloud; layer 1 from host, l>=2 on device
        nxx_row = persist.tile([1, N], F32, tag="nxx_row")
        nc.sync.dma_start(nxx_row[:], nxx_d[:, :])

        ecw = {}
        for name, C, O in ECL:
            for p in ("wu", "wv", "wvt"):
                shp = [1, O] if p == "wvt" else [C, O]
                tl = persist.tile(shp, F32, tag=f"{name}_{p}", name=f"{name}_{p}_sb")
                nc.sync.dma_start(tl[:], wd[f"{name}_{p}"][:, :])
                ecw[f"{name}_{p}"] = tl

        def feat_chunks(li):
            if li == 0:
                return [(xm1, 3)]
            return [(t, t.shape[0]) for t in yT[ECL[li - 1][0]]]

        # ================= edge conv layers =================
        for li, (name, C, O) in enumerate(ECL):
            if li > 0:
                with (
                    tc.tile_pool(name=f"xx_{name}", bufs=2) as xxp,
                    tc.tile_pool(name=f"xxps_{name}", bufs=2, space="PSUM") as xxps,
                ):
                    onescol = xxp.tile([128, 1], F32, tag="onescol")
                    nc.vector.memset(onescol[:], 1.0)
                    fcs = feat_chunks(li)
                    for cb in range(8):
                        ps = xxps.tile([1, 512], F32)
                        for ci, (t, nr) in enumerate(fcs):
                            sq = xxp.tile([128, 512], F32, tag="sq")
                            nc.scalar.activation(
                                sq[:nr, :], t[:nr, cb * 512 : (cb + 1) * 512], AF.Square
                            )
                            nc.tensor.matmul(
                                ps[:],
                                onescol[:nr, :],
                                sq[:nr, :],
                                start=(ci == 0),
                                stop=(ci == len(fcs) - 1),
                            )
                        sl = slice(cb * 512, (cb + 1) * 512)
                        nc.scalar.activation(nxx_row[0:1, sl], ps[:], AF.Copy, scale=-0.5)

            # ---- u for the full cloud -> u_dram ----
            with (
                tc.tile_pool(name=f"u_{name}", bufs=3) as up,
                tc.tile_pool(name=f"ups_{name}", bufs=2, space="PSUM") as ups,
            ):
                fcs = feat_chunks(li)
                for jt in range(NT_ALL):
                    ps = ups.tile([128, O], F32)
                    r0 = 0
                    for ci, (t, nr) in enumerate(fcs):
                        nc.tensor.matmul(
                            ps[:],
                            t[:nr, jt * 128 : (jt + 1) * 128],
                            ecw[f"{name}_wu"][r0 : r0 + nr, :],
                            start=(ci == 0),
                            stop=(ci == len(fcs) - 1),
                        )
                        r0 += nr
                    us = up.tile([128, O], F32, tag="us")
                    nc.scalar.activation(us[:], ps[:], AF.Copy)
                    nc.sync.dma_start(u_dram[name][jt * 128 : (jt + 1) * 128, :], us[:])

            with tc.tile_pool(name=f"lay_{name}", bufs=1) as lay:
                # ---- v' own half ----
                v_sb = lay.tile([128, NT_OWN * O], F32, tag="v_sb")
                with tc.tile_pool(name=f"vps_{name}", bufs=2, space="PSUM") as vps:
                    fcs = feat_chunks(li)
                    for qt in range(NT_OWN):
                        ps = vps.tile([128, O], F32)
                        r0 = 0
                        for t, nr in fcs:
                            nc.tensor.matmul(
                                ps[:],
                                t[:nr, qt * 128 : (qt + 1) * 128],
                                ecw[f"{name}_wv"][r0 : r0 + nr, :],
                                start=(r0 == 0),
                                stop=False,
                            )
                            r0 += nr
                        nc.tensor.matmul(
                            ps[:],
                            ones512[:, 0:128],
                            ecw[f"{name}_wvt"][:, :],
                            start=False,
                            stop=True,
                        )
                        nc.scalar.activation(v_sb[:, qt * O : (qt + 1) * O], ps[:], AF.Copy)

                # ---- distances + topk + gather + neighbour max ----
                ynat = lay.tile([128, NT_OWN * O], F32, tag="ynat")
                with (
                    tc.tile_pool(name=f"s_{name}", bufs=3, space="PSUM") as sps,
                    tc.tile_pool(name=f"tr_{name}", bufs=2, space="PSUM") as trp,
                    tc.tile_pool(name=f"tk_{name}", bufs=3) as tkp,
                    tc.tile_pool(name=f"ga_{name}", bufs=(2 if O == 256 else 3)) as gap,
                ):
                    fcs = feat_chunks(li)
                    for qt in range(NT_OWN):
                        # per-bank top-16 (values + bank-local indices), read
                        # straight from PSUM; round-0 match_replace evicts the
                        # bank row to SBUF for round 1.
                        bvals = tkp.tile([128, 8, 16], F32, tag="bvals")
                        bidx = tkp.tile([128, 8, 16], U16, tag="bidx")
                        for hb in range(4):  # 2-bank PSUM chunks
                            s_ps = sps.tile([128, N // 4], F32, tag="s_ps")
                            for cb in range(2):
                                for ci, (st, snr) in enumerate(fcs):
                                    nc.tensor.matmul(
                                        s_ps[:, cb * 512 : (cb + 1) * 512],
                                        dr(st[:snr, qt * 128 : (qt + 1) * 128]),
                                        dr(st[:snr, (hb * 2 + cb) * 512 : (hb * 2 + cb + 1) * 512]),
                                        start=(ci == 0),
                                        stop=False,
                                    )
                                nc.tensor.matmul(
                                    s_ps[:, cb * 512 : (cb + 1) * 512],
                                    dr(ones512[:, 0:128]),
                                    dr(nxx_row[:, (hb * 2 + cb) * 512 : (hb * 2 + cb + 1) * 512]),
                                    start=False,
                                    stop=True,
                                )
                            for cb in range(2):
                                b = hb * 2 + cb
                                row = tkp.tile([128, 512], F32, tag="row_sb")
                                nc.scalar.activation(
                                    row[:], s_ps[:, cb * 512 : (cb + 1) * 512], AF.Copy
                                )
                                v8 = bvals[:, b, 0:8]
                                i8 = bidx[:, b, 0:8]
                                nc.vector.max(out=v8, in_=row[:])
                                nc.vector.max_index(out=i8, in_max=v8, in_values=row[:])
                                nc.vector.match_replace(
                                    out=row[:], in_to_replace=v8, in_values=row[:],
                                    imm_value=-1e30,
                                )
                                v8b = bvals[:, b, 8:16]
                                i8b = bidx[:, b, 8:16]
                                nc.vector.max(out=v8b, in_=row[:])
                                nc.vector.max_index(out=i8b, in_max=v8b, in_values=row[:])
                        # global candidate arrays [128, 128]
                        idxf = tkp.tile([128, 8, 16], F32, tag="idxf")
                        nc.vector.tensor_copy(idxf[:], bidx[:])
                        for cb in range(8):
                            if cb:
                                nc.vector.tensor_scalar_add(
                                    idxf[:, cb, :], idxf[:, cb, :], float(cb * 512)
                                )
                        # top-24 values of the 128 candidates -> t20
                        mv = tkp.tile([128, 24], F32, tag="mv")
                        vwork = tkp.tile([128, 128], F32, tag="vwork")
                        bv_flat = bvals[:].rearrange("p a b -> p (a b)")
                        v8 = mv[:, 0:8]
                        nc.vector.max(out=v8, in_=bv_flat)
                        nc.vector.match_replace(
                            out=vwork[:], in_to_replace=v8, in_values=bv_flat,
                            imm_value=-1e30,
                        )
                        for r in (1, 2):
                            v8 = mv[:, r * 8 : (r + 1) * 8]
                            nc.vector.max(out=v8, in_=vwork[:])
                            if r < 2:
                                nc.vector.match_replace(
                                    out=vwork[:], in_to_replace=v8, in_values=vwork[:],
                                    imm_value=-1e30,
                                )
                        # seli = idxf + (bvals < t20) * -1e30   (t20 = mv[:, 19])
                        seli = tkp.tile([128, 128], F32, tag="seli")
                        nc.vector.tensor_scalar(
                            seli[:],
                            bv_flat,
                            mv[:, 19:20],
                            -1e30,
                            op0=ALU.is_lt,
                            op1=ALU.mult,
                        )
                        nc.vector.tensor_tensor(
                            out=seli[:],
                            in0=seli[:],
                            in1=idxf[:].rearrange("p a b -> p (a b)"),
                            op=ALU.add,
                        )
                        sidx = tkp.tile([128, 24], F32, tag="sidx")
                        for r in range(3):
                            v8 = sidx[:, r * 8 : (r + 1) * 8]
                            nc.vector.max(out=v8, in_=seli[:])
                            if r < 2:
                                nc.vector.match_replace(
                                    out=seli[:], in_to_replace=v8, in_values=seli[:],
                                    imm_value=-1e30,
                                )
                        idx32 = tkp.tile([128, K], U32, tag="idx32")
                        nc.vector.tensor_copy(idx32[:], sidx[:, 0:K])
                        gath = gap.tile([128, K, O], F32, tag="gath")
                        for j in range(K):
                            nc.gpsimd.indirect_dma_start(
                                out=gath[:, j, :],
                                out_offset=None,
                                in_=u_dram[name][:, :],
                                in_offset=IndirectOffsetOnAxis(
                                    ap=idx32[:, j : j + 1], axis=0
                                ),
                            )
                        m_sb = tkp.tile([128, O], F32, tag="m_sb")
                        nc.vector.tensor_reduce(
                            out=m_sb[:],
                            in_=gath[:].rearrange("p k o -> p o k"),
                            axis=AX.X,
                            op=ALU.max,
                        )
                        nc.vector.tensor_add(
                            out=m_sb[:], in0=m_sb[:], in1=v_sb[:, qt * O : (qt + 1) * O]
                        )
                        nc.scalar.activation(
                            ynat[:, qt * O : (qt + 1) * O], m_sb[:], AF.Prelu, alpha=0.2
                        )
                        # transpose this tile's own-half columns immediately so
                        # the transpose work overlaps later tiles' extraction
                        for oc in range(0, O, 128):
                            ow = min(128, O - oc)
                            tp = trp.tile([128, 128], F32)
                            nc.tensor.transpose(
                                tp[:ow, :],
                                ynat[:, qt * O + oc : qt * O + oc + ow],
                                ident_sb[:],
                            )
                            dst = yT[name][oc // 128]
                            dslice = dst[:ow, qt * 128 : (qt + 1) * 128]
                            if li == 3:
                                # y4 feeds only the MLP: store fp32r-rounded so
                                # the fuse/h1 matmuls can run in fp32r mode.
                                dslice = dslice.bitcast(F32R)
                            nc.scalar.activation(dslice, tp[:ow, :], AF.Copy)

            # ---- exchange halves (layers 1-3): partner = allreduce_sum - own ----
            if li < 3:
                ytl = yT[name]
                for ci_, t in enumerate(ytl):
                    nr = t.shape[0]
                    nc.sync.dma_start(
                        cc_in[name][ci_ * 128 : ci_ * 128 + nr, :], t[:nr, 0:NHALF]
                    )
                nc.gpsimd.collective_compute(
                    "AllReduce",
                    ALU.add,
                    replica_groups=RG,
                    ins=[cc_in[name][:]],
                    outs=[cc_out[name][:]],
                )
                with tc.tile_pool(name=f"ex_{name}", bufs=2) as exp_:
                    for ci_, t in enumerate(ytl):
                        nr = t.shape[0]
                        ssum = exp_.tile([128, NHALF], F32, tag="ssum")
                        nc.sync.dma_start(
                            ssum[:nr, :], cc_out[name][ci_ * 128 : ci_ * 128 + nr, :]
                        )
                        nc.vector.tensor_tensor(
                            out=t[:nr, NHALF:N],
                            in0=ssum[:nr, :],
                            in1=t[:nr, 0:NHALF],
                            op=ALU.subtract,
                        )

        # ================= MLP head (own half) =================
        xcat_chunks = [
            (yT["ec1"][0], 64),
            (yT["ec2"][0], 64),
            (yT["ec3"][0], 128),
            (yT["ec4"][0], 128),
            (yT["ec4"][1], 128),
        ]
        mlp = stack.enter_context(tc.tile_pool(name="mlp", bufs=1))

        g_sb = mlp.tile([128, 8], F32, tag="g_sb")
        with (
            tc.tile_pool(name="fw", bufs=2) as fwp,
            tc.tile_pool(name="fps", bufs=2, space="PSUM") as fps,
            tc.tile_pool(name="fsc", bufs=2) as fsc,
        ):
            for ec in range(8):
                fw_sb = []
                r0 = 0
                for wci, (t, nr) in enumerate(xcat_chunks):
                    wsb = fwp.tile([128, 128], F32, tag=f"fw_sb{wci}", name=f"fw_sb{wci}")
                    nc.sync.dma_start(
                        wsb[:nr, :], wd["fuse_w"][r0 : r0 + nr, ec * 128 : (ec + 1) * 128]
                    )
                    if wci >= 3:  # y4 chunks run in fp32r
                        wr = fwp.tile([128, 128], F32R, tag=f"fw_r{wci}", name=f"fw_r{wci}")
                        nc.vector.tensor_copy(wr[:nr, :], wsb[:nr, :])
                        wsb = wr
                    fw_sb.append((wsb, nr))
                    r0 += nr
                ft_sb = fwp.tile([1, 128], F32, tag="ft_sb")
                nc.sync.dma_start(ft_sb[:], wd["fuse_t"][:, ec * 128 : (ec + 1) * 128])
                for fb in range(4):
                    ps = fps.tile([128, 512], F32)
                    for ci, ((t, nr), (wsb, _)) in enumerate(zip(xcat_chunks, fw_sb)):
                        tt = t[:nr, fb * 512 : (fb + 1) * 512]
                        if ci >= 3:
                            tt = tt.bitcast(F32R)
                        nc.tensor.matmul(
                            ps[:], wsb[:nr, :], tt, start=(ci == 0), stop=False
                        )
                    nc.tensor.matmul(
                        ps[:], ft_sb[:, :], ones512[:, :], start=False, stop=True
                    )
                    fch = fsc.tile([128, 512], F32, tag="fch")
                    nc.scalar.activation(fch[:], ps[:], AF.Prelu, alpha=0.2)
                    red = fsc.tile([128, 1], F32, tag="red")
                    nc.vector.tensor_reduce(out=red[:], in_=fch[:], axis=AX.X, op=ALU.max)
                    if fb == 0:
                        nc.vector.tensor_copy(g_sb[:, ec : ec + 1], red[:])
                    else:
                        nc.vector.tensor_tensor(
                            out=g_sb[:, ec : ec + 1],
                            in0=g_sb[:, ec : ec + 1],
                            in1=red[:],
                            op=ALU.max,
                        )

        nc.sync.dma_start(g_in[:, :], g_sb[:])
        nc.gpsimd.collective_compute(
            "AllReduce", ALU.max, replica_groups=RG, ins=[g_in[:]], outs=[g_out[:]]
        )
        g2_sb = mlp.tile([128, 8], F32, tag="g2_sb")
        nc.sync.dma_start(g2_sb[:], g_out[:, :])

        bias_sb = mlp.tile([1, 512], F32, tag="bias_sb")
        with (
            tc.tile_pool(name="bw", bufs=2) as bwp,
            tc.tile_pool(name="bps", bufs=1, space="PSUM") as bps,
        ):
            ps = bps.tile([1, 512], F32)
            for kc in range(8):
                wsb = bwp.tile([128, 512], F32, tag="bw_sb")
                nc.sync.dma_start(wsb[:], wd["h1b_w"][kc * 128 : (kc + 1) * 128, :])
                nc.tensor.matmul(
                    ps[:], g2_sb[:, kc : kc + 1], wsb[:], start=(kc == 0), stop=False
                )
            wtb = bwp.tile([1, 512], F32, tag="bt_sb")
            nc.sync.dma_start(wtb[:], wd["h1b_w"][EMB : EMB + 1, :])
            nc.tensor.matmul(ps[:], one_cell[:], wtb[:], start=False, stop=True)
            nc.scalar.activation(bias_sb[:], ps[:], AF.Copy)

        h1T = [mlp.tile([128, NHALF], F32, tag=f"h1T_{i}", name=f"h1T_{i}") for i in range(4)]
        with (
            tc.tile_pool(name="h1w", bufs=2) as h1wp,
            tc.tile_pool(name="h1ps", bufs=2, space="PSUM") as h1ps,
        ):
            for cc in range(4):
                w_sb = []
                r0 = 0
                for wci, (t, nr) in enumerate(xcat_chunks):
                    wsb = h1wp.tile([128, 128], F32, tag=f"h1w_sb{wci}", name=f"h1w_sb{wci}")
                    nc.sync.dma_start(
                        wsb[:nr, :], wd["h1_w"][r0 : r0 + nr, cc * 128 : (cc + 1) * 128]
                    )
                    if wci >= 3:
                        wr = h1wp.tile([128, 128], F32R, tag=f"h1w_r{wci}", name=f"h1w_r{wci}")
                        nc.vector.tensor_copy(wr[:nr, :], wsb[:nr, :])
                        wsb = wr
                    w_sb.append((wsb, nr))
                    r0 += nr
                for fb in range(4):
                    ps = h1ps.tile([128, 512], F32)
                    for ci, ((t, nr), (wsb, _)) in enumerate(zip(xcat_chunks, w_sb)):
                        tt = t[:nr, fb * 512 : (fb + 1) * 512]
                        if ci >= 3:
                            tt = tt.bitcast(F32R)
                        nc.tensor.matmul(
                            ps[:], wsb[:nr, :], tt, start=(ci == 0), stop=False
                        )
                    nc.tensor.matmul(
                        ps[:],
                        bias_sb[:, cc * 128 : (cc + 1) * 128],
                        ones512[:, :],
                        start=False,
                        stop=True,
                    )
                    nc.scalar.activation(
                        h1T[cc][:, fb * 512 : (fb + 1) * 512].bitcast(F32R),
                        ps[:],
                        AF.Prelu,
                        alpha=0.2,
                    )

        h2T = [mlp.tile([128, NHALF], F32, tag=f"h2T_{i}", name=f"h2T_{i}") for i in range(2)]
        with (
            tc.tile_pool(name="h2w", bufs=2) as h2wp,
            tc.tile_pool(name="h2ps", bufs=2, space="PSUM") as h2ps,
        ):
            for cc in range(2):
                w_sb = []
                for kc in range(4):
                    wst = h2wp.tile([128, 128], F32, tag=f"h2w_st{kc}", name=f"h2w_st{kc}")
                    nc.sync.dma_start(
                        wst[:],
                        wd["h2_w"][kc * 128 : (kc + 1) * 128, cc * 128 : (cc + 1) * 128],
                    )
                    wsb = h2wp.tile([128, 128], F32R, tag=f"h2w_sb{kc}", name=f"h2w_sb{kc}")
                    nc.vector.tensor_copy(wsb[:], wst[:])
                    w_sb.append(wsb)
                t_sb = h2wp.tile([1, 128], F32, tag="h2t_sb")
                nc.sync.dma_start(t_sb[:], wd["h2_t"][:, cc * 128 : (cc + 1) * 128])
                for fb in range(4):
                    ps = h2ps.tile([128, 512], F32)
                    for kc in range(4):
                        nc.tensor.matmul(
                            ps[:],
                            w_sb[kc][:, :],
                            h1T[kc][:, fb * 512 : (fb + 1) * 512].bitcast(F32R),
                            start=(kc == 0),
                            stop=False,
                        )
                    nc.tensor.matmul(
                        ps[:], t_sb[:, :], ones512[:, :], start=False, stop=True
                    )
                    nc.scalar.activation(
                        h2T[cc][:, fb * 512 : (fb + 1) * 512], ps[:], AF.Prelu, alpha=0.2
                    )

        with (
            tc.tile_pool(name="h3w", bufs=1) as h3wp,
            tc.tile_pool(name="h3ps", bufs=2, space="PSUM") as h3ps,
            tc.tile_pool(name="h3o", bufs=3) as h3op,
        ):
            w_sb = h3wp.tile([128, 2, NCLS], F32, tag="h3w_sb")
            nc.sync.dma_start(w_sb[:, 0, :], wd["h3_w"][0:128, :])
            nc.sync.dma_start(w_sb[:, 1, :], wd["h3_w"][128:256, :])
            t_sb = h3wp.tile([1, NCLS], F32, tag="h3t_sb")
            nc.sync.dma_start(t_sb[:], wd["h3_t"][:, :])
            for qt in range(NT_OWN):
                ps = h3ps.tile([128, NCLS], F32)
                for kc in range(2):
                    nc.tensor.matmul(
                        ps[:],
                        h2T[kc][:, qt * 128 : (qt + 1) * 128],
                        w_sb[:, kc, :],
                        start=(kc == 0),
                        stop=False,
                    )
                nc.tensor.matmul(
                    ps[:], ones512[:, 0:128], t_sb[:, :], start=False, stop=True
                )
                osb = h3op.tile([128, NCLS], F32, tag="osb")
                nc.scalar.activation(osb[:], ps[:], AF.Copy)
                nc.sync.dma_start(out_d[qt * 128 : (qt + 1) * 128, :], osb[:])

    split_multiwait(nc)
    return nc


def kernel(xyz, params, trace=False):
    import os

    import concourse.bass_utils as bass_utils

    xyz = np.asarray(xyz, np.float32)
    if "prog" not in _PROG_CACHE:
        _PROG_CACHE["prog"] = build_program()
    nc = _PROG_CACHE["prog"]

    wshared = host_weights(params)
    core_ids = list(range(8))
    in_maps = []
    for c in core_ids:
        m = dict(wshared)
        m.update(host_core_inputs(xyz, c))
        in_maps.append(m)
    res = bass_utils.run_bass_kernel_spmd(
        nc, in_maps, core_ids, trace=trace or bool(os.environ.get("KTRACE"))
    )
    _PROG_CACHE["last_res"] = res
    out = np.zeros((B, N, NCLS), np.float32)
    for c in core_ids:
        b, h = c // 2, c % 2
        out[b, h * NHALF : (h + 1) * NHALF, :] = res.results[c]["out"]
    return out


# revision 39
# speedup vs baseline: 1.1812x; 1.0732x over previous
"""DGCNN segmentation Bass kernel for 8 trn2 NeuronCores.

Sharding: data-parallel over (cloud b, half h): core c = 2*b + h owns
queries [h*2048:(h+1)*2048] of cloud b. Candidates are kept in a
core-relative order (own half first) so the SPMD program is identical on
every core; halves are exchanged per layer with a pairwise AllReduce(add)
(partner = sum - own).

EdgeConv is linearized: W @ [xj - xi; xi] = W1 @ xj + (W2 - W1) @ xi, and
eval-mode BN is folded into the weights on the host. LeakyReLU(0.2) is the
ACT engine's Prelu. Per-query top-20 neighbours come from 3 rounds of
max8/max_index/match_replace on the negative half-distance row; neighbour
feature rows are gathered with per-partition indirect DMA and max-reduced.
"""
import sys

if "/opt/trn_rl_repo" not in sys.path:
    sys.path.insert(0, "/opt/trn_rl_repo")

import numpy as np

EPS = 1e-5
_WS_CTR = [0]


def split_multiwait(nc, max_waits=1):
    """This walrus build rejects >1 sync-wait per instruction; hoist extras
    onto NoOp wait-carriers inserted before the instruction."""
    import concourse.mybir as mybir

    nsplit = 0
    for _, bbwrap in nc.bb_map.items():
        bb = bbwrap.bb
        newlist = []
        changed = False
        for inst in bb.instructions:
            si = inst.sync_info
            waits = list(si.on_wait) if si is not None and si.on_wait else []
            if len(waits) > max_waits:
                head = waits[:-max_waits]
                si.on_wait = waits[-max_waits:]
                for c0 in range(0, len(head), max_waits):
                    _WS_CTR[0] += 1
                    nop = mybir.InstNoOp(name=f"I-wsplit-{_WS_CTR[0]}", ins=[], outs=[])
                    nop.engine = inst.engine
                    nop.sync_info = mybir.SyncInfo(
                        on_wait=list(head[c0 : c0 + max_waits]), on_update=[]
                    )
                    nc.register_instruction(nop)
                    newlist.append(nop)
                changed = True
                nsplit += 1
            newlist.append(inst)
        if changed:
            bb.instructions = newlist
    return nsplit
B, N, K, NCLS, EMB = 4, 4096, 20, 50, 1024
NHALF = N // 2  # 2048
NT_OWN = NHALF // 128  # 16 query tiles per core
NT_ALL = N // 128  # 32
ECL = [
    # (name, C_in, O)
    ("ec1", 3, 64),
    ("ec2", 64, 64),
    ("ec3", 64, 128),
    ("ec4", 128, 256),
]


def _fold_bn(p):
    s = np.asarray(p["g"], np.float32) / np.sqrt(np.asarray(p["v"], np.float32) + EPS)
    t = np.asarray(p["b"], np.float32) - np.asarray(p["m"], np.float32) * s
    return s, t


def host_weights(params):
    """Folded weight tensors, shared across cores."""
    w = {}
    for name, C, O in ECL:
        p = params[name]
        pw = np.asarray(p["w"], np.float32)  # [O, 2C]
        s, t = _fold_bn(p)
        W1 = pw[:, :C]
        W2 = pw[:, C:]
        w[f"{name}_wu"] = np.ascontiguousarray((s[:, None] * W1).T)  # [C, O]
        w[f"{name}_wv"] = np.ascontiguousarray((s[:, None] * (W2 - W1)).T)  # [C, O]
        w[f"{name}_wvt"] = np.ascontiguousarray(t[None, :])  # [1, O]
    s, t = _fold_bn(params["fuse"])
    fw = (s[:, None] * np.asarray(params["fuse"]["w"], np.float32)).T  # [512, 1024]
    w["fuse_w"] = np.ascontiguousarray(fw)
    w["fuse_t"] = np.ascontiguousarray(t[None, :])  # [1, 1024]
    s, t = _fold_bn(params["h1"])
    h1w = s[:, None] * np.asarray(params["h1"]["w"], np.float32)  # [512, 1536]
    w["h1_w"] = np.ascontiguousarray(h1w[:, :512].T)  # [512 xcat, 512 ch]
    w["h1b_w"] = np.ascontiguousarray(
        np.concatenate([h1w[:, 512:].T, t[None, :]], axis=0)
    )  # [1025, 512]: bias' = h1b_w.T @ [g; 1]
    s, t = _fold_bn(params["h2"])
    w["h2_w"] = np.ascontiguousarray(
        (s[:, None] * np.asarray(params["h2"]["w"], np.float32)).T
    )  # [512, 256]
    w["h2_t"] = np.ascontiguousarray(t[None, :])  # [1, 256]
    w["h3_w"] = np.ascontiguousarray(np.asarray(params["h3"]["w"], np.float32).T)  # [256, 50]
    w["h3_t"] = np.ascontiguousarray(np.asarray(params["h3"]["bias"], np.float32)[None, :])
    w["ident"] = np.eye(128, dtype=np.float32)
    return w


def host_core_inputs(xyz, c):
    """Per-core layer-1 feature tensors in core-relative (own-half-first) order.

    The distance score is s = x_q . x_c - xx_c/2 (the query-side xx term is a
    per-row constant and cannot change that row's top-k)."""
    h = c % 2
    x = np.asarray(xyz[c // 2], np.float32)  # [4096, 3]
    perm = np.concatenate(
        [
            np.arange(h * NHALF, (h + 1) * NHALF),
            np.arange((1 - h) * NHALF, (2 - h) * NHALF) % N,
        ]
    )
    xp = x[perm]
    nxx = -0.5 * (xp * xp).sum(-1)
    return {
        "xT": np.ascontiguousarray(xp.T),
        "nxx": np.ascontiguousarray(nxx[None, :]),
    }


_PROG_CACHE = {}


def build_program():
    import contextlib

    import concourse.bass as bass
    import concourse.mybir as mybir
    from concourse.bass import IndirectOffsetOnAxis
    from concourse.tile import TileContext

    import os
    F32 = mybir.dt.float32
    F32R = mybir.dt.float32r
    DIST_R = bool(os.environ.get("KDIST_F32R"))
    MLP_R = bool(os.environ.get("KMLP_F32R"))
    def dr(ap):
        return ap.bitcast(F32R) if DIST_R else ap
    def mr_(ap):
        return ap.bitcast(F32R) if MLP_R else ap
    U16 = mybir.dt.uint16
    U32 = mybir.dt.uint32
    AF = mybir.ActivationFunctionType
    ALU = mybir.AluOpType
    AX = mybir.AxisListType
    RG = [[0, 1], [2, 3], [4, 5], [6, 7]]

    nc = bass.Bass("TRN2", target_bir_lowering=False, debug=False)

    xT_d = nc.declare_dram_parameter("xT", [3, N], F32, isOutput=False)
    nxx_d = nc.declare_dram_parameter("nxx", [1, N], F32, isOutput=False)
    wd = {}
    for name, C, O in ECL:
        wd[f"{name}_wu"] = nc.declare_dram_parameter(f"{name}_wu", [C, O], F32, isOutput=False)
        wd[f"{name}_wv"] = nc.declare_dram_parameter(f"{name}_wv", [C, O], F32, isOutput=False)
        wd[f"{name}_wvt"] = nc.declare_dram_parameter(f"{name}_wvt", [1, O], F32, isOutput=False)
    wd["fuse_w"] = nc.declare_dram_parameter("fuse_w", [512, EMB], F32, isOutput=False)
    wd["fuse_t"] = nc.declare_dram_parameter("fuse_t", [1, EMB], F32, isOutput=False)
    wd["h1_w"] = nc.declare_dram_parameter("h1_w", [512, 512], F32, isOutput=False)
    wd["h1b_w"] = nc.declare_dram_parameter("h1b_w", [EMB + 1, 512], F32, isOutput=False)
    wd["h2_w"] = nc.declare_dram_parameter("h2_w", [512, 256], F32, isOutput=False)
    wd["h2_t"] = nc.declare_dram_parameter("h2_t", [1, 256], F32, isOutput=False)
    wd["h3_w"] = nc.declare_dram_parameter("h3_w", [256, NCLS], F32, isOutput=False)
    wd["h3_t"] = nc.declare_dram_parameter("h3_t", [1, NCLS], F32, isOutput=False)
    wd["ident"] = nc.declare_dram_parameter("ident", [128, 128], F32, isOutput=False)
    out_d = nc.declare_dram_parameter("out", [NHALF, NCLS], F32, isOutput=True)

    import os as _os
    _pam = "stack" if _os.environ.get("KSTACK") else "queue"
    with TileContext(nc, num_cores=8, pool_alloc_mode=_pam) as tc, contextlib.ExitStack() as stack:
        persist = stack.enter_context(tc.tile_pool(name="persist", bufs=1))
        dramp = stack.enter_context(tc.tile_pool(name="dramp", bufs=1, space="DRAM"))
        u_dram = {
            name: dramp.tile([N, O], F32, tag=f"u_{name}", name=f"u_{name}")
            for name, C, O in ECL
        }
        cc_in = {}
        cc_out = {}
        for li, (name, C, O) in enumerate(ECL[:3]):
            cc_in[name] = dramp.tile([O, NHALF], F32, tag=f"cc_in_{name}", name=f"cc_in_{name}")
            cc_out[name] = dramp.tile([O, NHALF], F32, tag=f"cc_out_{name}", name=f"cc_out_{name}")
        g_in = dramp.tile([128, 8], F32, tag="g_in", name="g_in")
        g_out = dramp.tile([128, 8], F32, tag="g_out", name="g_out")

        ones512 = persist.tile([1, 512], F32, tag="ones512")
        nc.vector.memset(ones512[:], 1.0)
        one_cell = persist.tile([1, 1], F32, tag="one_cell")
        nc.vector.memset(one_cell[:], 1.0)
        # constant bank offsets [128, 4, 16]: column group cb holds cb*1024
        bankoff = persist.tile([128, 4, 16], F32, tag="bankoff")
        for _cb in range(4):
            nc.vector.memset(bankoff[:, _cb, :], float(_cb * 1024))
        ident_sb = persist.tile([128, 128], F32, tag="ident")
        nc.sync.dma_start(ident_sb[:], wd["ident"][:, :])

        # transposed feature tensors; y1..y3 full cloud, y4 own half only
        yT = {
            "ec1": [persist.tile([64, N], F32, tag="y1T", name="y1T")],
            "ec2": [persist.tile([64, N], F32, tag="y2T", name="y2T")],
            "ec3": [persist.tile([128, N], F32, tag="y3T", name="y3T")],
            "ec4": [
                persist.tile([128, NHALF], F32, tag="y4Ta", name="y4Ta"),
                persist.tile([128, NHALF], F32, tag="y4Tb", name="y4Tb"),
            ],
        }
        xm1 = persist.tile([3, N], F32, tag="xm1")
        nc.sync.dma_start(xm1[:], xT_d[:, :])

        # -xx/2 row of the candidate cloud; layer 1 from host, l>=2 on device
        nxx_row = persist.tile([1, N], F32, tag="nxx_row")
        nc.sync.dma_start(nxx_row[:], nxx_d[:, :])

        ecw = {}
        for name, C, O in ECL:
            for p in ("wu", "wv", "wvt"):
                shp = [1, O] if p == "wvt" else [C, O]
                tl = persist.tile(shp, F32, tag=f"{name}_{p}", name=f"{name}_{p}_sb")
                nc.sync.dma_start(tl[:], wd[f"{name}_{p}"][:, :])
                ecw[f"{name}_{p}"] = tl

        def feat_chunks(li):
            if li == 0:
                return [(xm1, 3)]
            return [(t, t.shape[0]) for t in yT[ECL[li - 1][0]]]

        # ================= edge conv layers =================
        for li, (name, C, O) in enumerate(ECL):
            if li > 0:
                with (
                    tc.tile_pool(name=f"xx_{name}", bufs=2) as xxp,
                    tc.tile_pool(name=f"xxps_{name}", bufs=2, space="PSUM") as xxps,
                ):
                    onescol = xxp.tile([128, 1], F32, tag="onescol")
                    nc.vector.memset(onescol[:], 1.0)
                    fcs = feat_chunks(li)
                    for cb in range(8):
                        ps = xxps.tile([1, 512], F32)
                        for ci, (t, nr) in enumerate(fcs):
                            sq = xxp.tile([128, 512], F32, tag="sq")
                            nc.scalar.activation(
                                sq[:nr, :], t[:nr, cb * 512 : (cb + 1) * 512], AF.Square
                            )
                            nc.tensor.matmul(
                                ps[:],
                                onescol[:nr, :],
                                sq[:nr, :],
                                start=(ci == 0),
                                stop=(ci == len(fcs) - 1),
                            )
                        sl = slice(cb * 512, (cb + 1) * 512)
                        nc.scalar.activation(nxx_row[0:1, sl], ps[:], AF.Copy, scale=-0.5)

            # ---- u for the full cloud -> u_dram ----
            with (
                tc.tile_pool(name=f"u_{name}", bufs=3) as up,
                tc.tile_pool(name=f"ups_{name}", bufs=2, space="PSUM") as ups,
            ):
                fcs = feat_chunks(li)
                for jt in range(NT_ALL):
                    ps = ups.tile([128, O], F32)
                    r0 = 0
                    for ci, (t, nr) in enumerate(fcs):
                        nc.tensor.matmul(
                            ps[:],
                            t[:nr, jt * 128 : (jt + 1) * 128],
                            ecw[f"{name}_wu"][r0 : r0 + nr, :],
                            start=(ci == 0),
                            stop=(ci == len(fcs) - 1),
                        )
                        r0 += nr
                    us = up.tile([128, O], F32, tag="us")
                    nc.scalar.activation(us[:], ps[:], AF.Copy)
                    nc.sync.dma_start(u_dram[name][jt * 128 : (jt + 1) * 128, :], us[:])

            with tc.tile_pool(name=f"lay_{name}", bufs=1) as lay:
                # ---- v' own half ----
                v_sb = lay.tile([128, NT_OWN * O], F32, tag="v_sb")
                with tc.tile_pool(name=f"vps_{name}", bufs=2, space="PSUM") as vps:
                    fcs = feat_chunks(li)
                    for qt in range(NT_OWN):
                        ps = vps.tile([128, O], F32)
                        r0 = 0
                        for t, nr in fcs:
                            nc.tensor.matmul(
                                ps[:],
                                t[:nr, qt * 128 : (qt + 1) * 128],
                                ecw[f"{name}_wv"][r0 : r0 + nr, :],
                                start=(r0 == 0),
                                stop=False,
                            )
                            r0 += nr
                        nc.tensor.matmul(
                            ps[:],
                            ones512[:, 0:128],
                            ecw[f"{name}_wvt"][:, :],
                            start=False,
                            stop=True,
                        )
                        nc.scalar.activation(v_sb[:, qt * O : (qt + 1) * O], ps[:], AF.Copy)

                # ---- distances + topk + gather + neighbour max ----
                ynat = lay.tile([128, NT_OWN * O], F32, tag="ynat")
                with (
                    tc.tile_pool(name=f"s_{name}", bufs=3, space="PSUM") as sps,
                    tc.tile_pool(name=f"tr_{name}", bufs=2, space="PSUM") as trp,
                    tc.tile_pool(name=f"tk_{name}", bufs=3) as tkp,
                    tc.tile_pool(name=f"ga_{name}", bufs=(2 if O == 256 else 3)) as gap,
                ):
                    fcs = feat_chunks(li)
                    for qt in range(NT_OWN):
                        # per-bank top-16 (values + bank-local indices), read
                        # straight from PSUM; round-0 match_replace evicts the
                        # bank row to SBUF for round 1.
                        bvals = tkp.tile([128, 4, 16], F32, tag="bvals")
                        bidx = tkp.tile([128, 4, 16], U16, tag="bidx")
                        for hb in range(4):  # 2-bank PSUM chunks
                            s_ps = sps.tile([128, N // 4], F32, tag="s_ps")
                            for cb in range(2):
                                for ci, (st, snr) in enumerate(fcs):
                                    nc.tensor.matmul(
                                        s_ps[:, cb * 512 : (cb + 1) * 512],
                                        dr(st[:snr, qt * 128 : (qt + 1) * 128]),
                                        dr(st[:snr, (hb * 2 + cb) * 512 : (hb * 2 + cb + 1) * 512]),
                                        start=(ci == 0),
                                        stop=False,
                                    )
                                nc.tensor.matmul(
                                    s_ps[:, cb * 512 : (cb + 1) * 512],
                                    dr(ones512[:, 0:128]),
                                    dr(nxx_row[:, (hb * 2 + cb) * 512 : (hb * 2 + cb + 1) * 512]),
                                    start=False,
                                    stop=True,
                                )
                            # extract top-16 of the whole 1024-wide chunk
                            # (P(>16 of the top-20 in one of 4 banks) ~ 1e-8)
                            row = tkp.tile([128, 1024], F32, tag="row_sb")
                            nc.scalar.activation(row[:], s_ps[:], AF.Copy)
                            v8 = bvals[:, hb, 0:8]
                            i8 = bidx[:, hb, 0:8]
                            nc.vector.max(out=v8, in_=row[:])
                            nc.vector.max_index(out=i8, in_max=v8, in_values=row[:])
                            nc.vector.match_replace(
                                out=row[:], in_to_replace=v8, in_values=row[:],
                                imm_value=-1e30,
                            )
                            v8b = bvals[:, hb, 8:16]
                            i8b = bidx[:, hb, 8:16]
                            nc.vector.max(out=v8b, in_=row[:])
                            nc.vector.max_index(out=i8b, in_max=v8b, in_values=row[:])
                        # global candidate arrays [128, 64]
                        idxf = tkp.tile([128, 4, 16], F32, tag="idxf")
                        nc.vector.tensor_copy(idxf[:], bidx[:])
                        nc.vector.tensor_tensor(
                            out=idxf[:], in0=idxf[:], in1=bankoff[:], op=ALU.add
                        )
                        # top-24 values of the 128 candidates -> t20
                        mv = tkp.tile([128, 24], F32, tag="mv")
                        vwork = tkp.tile([128, 64], F32, tag="vwork")
                        bv_flat = bvals[:].rearrange("p a b -> p (a b)")
                        v8 = mv[:, 0:8]
                        nc.vector.max(out=v8, in_=bv_flat)
                        nc.vector.match_replace(
                            out=vwork[:], in_to_replace=v8, in_values=bv_flat,
                            imm_value=-1e30,
                        )
                        for r in (1, 2):
                            v8 = mv[:, r * 8 : (r + 1) * 8]
                            nc.vector.max(out=v8, in_=vwork[:])
                            if r < 2:
                                nc.vector.match_replace(
                                    out=vwork[:], in_to_replace=v8, in_values=vwork[:],
                                    imm_value=-1e30,
                                )
                        # seli = idxf + (bvals < t20) * -1e30   (t20 = mv[:, 19])
                        seli = tkp.tile([128, 64], F32, tag="seli")
                        nc.vector.tensor_scalar(
                            seli[:],
                            bv_flat,
                            mv[:, 19:20],
                            -1e30,
                            op0=ALU.is_lt,
                            op1=ALU.mult,
                        )
                        nc.vector.tensor_tensor(
                            out=seli[:],
                            in0=seli[:],
                            in1=idxf[:].rearrange("p a b -> p (a b)"),
                            op=ALU.add,
                        )
                        sidx = tkp.tile([128, 24], F32, tag="sidx")
                        for r in range(3):
                            v8 = sidx[:, r * 8 : (r + 1) * 8]
                            nc.vector.max(out=v8, in_=seli[:])
                            if r < 2:
                                nc.vector.match_replace(
                                    out=seli[:], in_to_replace=v8, in_values=seli[:],
                                    imm_value=-1e30,
                                )
                        idx32 = tkp.tile([128, K], U32, tag="idx32")
                        nc.vector.tensor_copy(idx32[:], sidx[:, 0:K])
                        gath = gap.tile([128, K, O], F32, tag="gath")
                        for j in range(K):
                            nc.gpsimd.indirect_dma_start(
                                out=gath[:, j, :],
                                out_offset=None,
                                in_=u_dram[name][:, :],
                                in_offset=IndirectOffsetOnAxis(
                                    ap=idx32[:, j : j + 1], axis=0
                                ),
                            )
                        m_sb = tkp.tile([128, O], F32, tag="m_sb")
                        nc.vector.tensor_reduce(
                            out=m_sb[:],
                            in_=gath[:].rearrange("p k o -> p o k"),
                            axis=AX.X,
                            op=ALU.max,
                        )
                        nc.vector.tensor_add(
                            out=m_sb[:], in0=m_sb[:], in1=v_sb[:, qt * O : (qt + 1) * O]
                        )
                        nc.scalar.activation(
                            ynat[:, qt * O : (qt + 1) * O], m_sb[:], AF.Prelu, alpha=0.2
                        )
                        # transpose this tile's own-half columns immediately so
                        # the transpose work overlaps later tiles' extraction
                        for oc in range(0, O, 128):
                            ow = min(128, O - oc)
                            tp = trp.tile([128, 128], F32)
                            nc.tensor.transpose(
                                tp[:ow, :],
                                ynat[:, qt * O + oc : qt * O + oc + ow],
                                ident_sb[:],
                            )
                            dst = yT[name][oc // 128]
                            dslice = dst[:ow, qt * 128 : (qt + 1) * 128]
                            if li == 3:
                                # y4 feeds only the MLP: store fp32r-rounded so
                                # the fuse/h1 matmuls can run in fp32r mode.
                                dslice = dslice.bitcast(F32R)
                            nc.scalar.activation(dslice, tp[:ow, :], AF.Copy)

            # ---- exchange halves (layers 1-3): partner = allreduce_sum - own ----
            if li < 3:
                ytl = yT[name]
                for ci_, t in enumerate(ytl):
                    nr = t.shape[0]
                    nc.sync.dma_start(
                        cc_in[name][ci_ * 128 : ci_ * 128 + nr, :], t[:nr, 0:NHALF]
                    )
                nc.gpsimd.collective_compute(
                    "AllReduce",
                    ALU.add,
                    replica_groups=RG,
                    ins=[cc_in[name][:]],
                    outs=[cc_out[name][:]],
                )
                with tc.tile_pool(name=f"ex_{name}", bufs=2) as exp_:
                    for ci_, t in enumerate(ytl):
                        nr = t.shape[0]
                        ssum = exp_.tile([128, NHALF], F32, tag="ssum")
                        nc.sync.dma_start(
                            ssum[:nr, :], cc_out[name][ci_ * 128 : ci_ * 128 + nr, :]
                        )
                        nc.vector.tensor_tensor(
                            out=t[:nr, NHALF:N],
                            in0=ssum[:nr, :],
                            in1=t[:nr, 0:NHALF],
                            op=ALU.subtract,
                        )

        # ================= MLP head (own half) =================
        xcat_chunks = [
            (yT["ec1"][0], 64),
            (yT["ec2"][0], 64),
            (yT["ec3"][0], 128),
            (yT["ec4"][0], 128),
            (yT["ec4"][1], 128),
        ]
        mlp = stack.enter_context(tc.tile_pool(name="mlp", bufs=1))

        g_sb = mlp.tile([128, 8], F32, tag="g_sb")
        with (
            tc.tile_pool(name="fw", bufs=2) as fwp,
            tc.tile_pool(name="fps", bufs=2, space="PSUM") as fps,
            tc.tile_pool(name="fsc", bufs=2) as fsc,
        ):
            for ec in range(8):
                fw_sb = []
                r0 = 0
                for wci, (t, nr) in enumerate(xcat_chunks):
                    wsb = fwp.tile([128, 128], F32, tag=f"fw_sb{wci}", name=f"fw_sb{wci}")
                    nc.sync.dma_start(
                        wsb[:nr, :], wd["fuse_w"][r0 : r0 + nr, ec * 128 : (ec + 1) * 128]
                    )
                    if wci >= 3:  # y4 chunks run in fp32r
                        wr = fwp.tile([128, 128], F32R, tag=f"fw_r{wci}", name=f"fw_r{wci}")
                        nc.vector.tensor_copy(wr[:nr, :], wsb[:nr, :])
                        wsb = wr
                    fw_sb.append((wsb, nr))
                    r0 += nr
                ft_sb = fwp.tile([1, 128], F32, tag="ft_sb")
                nc.sync.dma_start(ft_sb[:], wd["fuse_t"][:, ec * 128 : (ec + 1) * 128])
                for fb in range(4):
                    ps = fps.tile([128, 512], F32)
                    for ci, ((t, nr), (wsb, _)) in enumerate(zip(xcat_chunks, fw_sb)):
                        tt = t[:nr, fb * 512 : (fb + 1) * 512]
                        if ci >= 3:
                            tt = tt.bitcast(F32R)
                        nc.tensor.matmul(
                            ps[:], wsb[:nr, :], tt, start=(ci == 0), stop=False
                        )
                    nc.tensor.matmul(
                        ps[:], ft_sb[:, :], ones512[:, :], start=False, stop=True
                    )
                    fch = fsc.tile([128, 512], F32, tag="fch")
                    nc.scalar.activation(fch[:], ps[:], AF.Prelu, alpha=0.2)
                    red = fsc.tile([128, 1], F32, tag="red")
                    nc.vector.tensor_reduce(out=red[:], in_=fch[:], axis=AX.X, op=ALU.max)
                    if fb == 0:
                        nc.vector.tensor_copy(g_sb[:, ec : ec + 1], red[:])
                    else:
                        nc.vector.tensor_tensor(
                            out=g_sb[:, ec : ec + 1],
                            in0=g_sb[:, ec : ec + 1],
                            in1=red[:],
                            op=ALU.max,
                        )

        nc.sync.dma_start(g_in[:, :], g_sb[:])
        nc.gpsimd.collective_compute(
            "AllReduce", ALU.max, replica_groups=RG, ins=[g_in[:]], outs=[g_out[:]]
        )
        g2_sb = mlp.tile([128, 8], F32, tag="g2_sb")
        nc.sync.dma_start(g2_sb[:], g_out[:, :])

        bias_sb = mlp.tile([1, 512], F32, tag="bias_sb")
        with (
            tc.tile_pool(name="bw", bufs=2) as bwp,
            tc.tile_pool(name="bps", bufs=1, space="PSUM") as bps,
        ):
            ps = bps.tile([1, 512], F32)
            for kc in range(8):
                wsb = bwp.tile([128, 512], F32, tag="bw_sb")
                nc.sync.dma_start(wsb[:], wd["h1b_w"][kc * 128 : (kc + 1) * 128, :])
                nc.tensor.matmul(
                    ps[:], g2_sb[:, kc : kc + 1], wsb[:], start=(kc == 0), stop=False
                )
            wtb = bwp.tile([1, 512], F32, tag="bt_sb")
            nc.sync.dma_start(wtb[:], wd["h1b_w"][EMB : EMB + 1, :])
            nc.tensor.matmul(ps[:], one_cell[:], wtb[:], start=False, stop=True)
            nc.scalar.activation(bias_sb[:], ps[:], AF.Copy)

        h1T = [mlp.tile([128, NHALF], F32, tag=f"h1T_{i}", name=f"h1T_{i}") for i in range(4)]
        with (
            tc.tile_pool(name="h1w", bufs=2) as h1wp,
            tc.tile_pool(name="h1ps", bufs=2, space="PSUM") as h1ps,
        ):
            for cc in range(4):
                w_sb = []
                r0 = 0
                for wci, (t, nr) in enumerate(xcat_chunks):
                    wsb = h1wp.tile([128, 128], F32, tag=f"h1w_sb{wci}", name=f"h1w_sb{wci}")
                    nc.sync.dma_start(
                        wsb[:nr, :], wd["h1_w"][r0 : r0 + nr, cc * 128 : (cc + 1) * 128]
                    )
                    if wci >= 3:
                        wr = h1wp.tile([128, 128], F32R, tag=f"h1w_r{wci}", name=f"h1w_r{wci}")
                        nc.vector.tensor_copy(wr[:nr, :], wsb[:nr, :])
                        wsb = wr
                    w_sb.append((wsb, nr))
                    r0 += nr
                for fb in range(4):
                    ps = h1ps.tile([128, 512], F32)
                    for ci, ((t, nr), (wsb, _)) in enumerate(zip(xcat_chunks, w_sb)):
                        tt = t[:nr, fb * 512 : (fb + 1) * 512]
                        if ci >= 3:
                            tt = tt.bitcast(F32R)
                        nc.tensor.matmul(
                            ps[:], wsb[:nr, :], tt, start=(ci == 0), stop=False
                        )
                    nc.tensor.matmul(
                        ps[:],
                        bias_sb[:, cc * 128 : (cc + 1) * 128],
                        ones512[:, :],
                        start=False,
                        stop=True,
                    )
                    nc.scalar.activation(
                        h1T[cc][:, fb * 512 : (fb + 1) * 512].bitcast(F32R),
                        ps[:],
                        AF.Prelu,
                        alpha=0.2,
                    )

        h2T = [mlp.tile([128, NHALF], F32, tag=f"h2T_{i}", name=f"h2T_{i}") for i in range(2)]
        with (
            tc.tile_pool(name="h2w", bufs=2) as h2wp,
            tc.tile_pool(name="h2ps", bufs=2, space="PSUM") as h2ps,
        ):
            for cc in range(2):
                w_sb = []
                for kc in range(4):
                    wst = h2wp.tile([128, 128], F32, tag=f"h2w_st{kc}", name=f"h2w_st{kc}")
                    nc.sync.dma_start(
                        wst[:],
                        wd["h2_w"][kc * 128 : (kc + 1) * 128, cc * 128 : (cc + 1) * 128],
                    )
                    wsb = h2wp.tile([128, 128], F32R, tag=f"h2w_sb{kc}", name=f"h2w_sb{kc}")
                    nc.vector.tensor_copy(wsb[:], wst[:])
                    w_sb.append(wsb)
                t_sb = h2wp.tile([1, 128], F32, tag="h2t_sb")
                nc.sync.dma_start(t_sb[:], wd["h2_t"][:, cc * 128 : (cc + 1) * 128])
                for fb in range(4):
                    ps = h2ps.tile([128, 512], F32)
                    for kc in range(4):
                        nc.tensor.matmul(
                            ps[:],
                            w_sb[kc][:, :],
                            h1T[kc][:, fb * 512 : (fb + 1) * 512].bitcast(F32R),
                            start=(kc == 0),
                            stop=False,
                        )
                    nc.tensor.matmul(
                        ps[:], t_sb[:, :], ones512[:, :], start=False, stop=True
                    )
                    nc.scalar.activation(
                        h2T[cc][:, fb * 512 : (fb + 1) * 512], ps[:], AF.Prelu, alpha=0.2
                    )

        with (
            tc.tile_pool(name="h3w", bufs=1) as h3wp,
            tc.tile_pool(name="h3ps", bufs=2, space="PSUM") as h3ps,
            tc.tile_pool(name="h3o", bufs=3) as h3op,
        ):
            w_sb = h3wp.tile([128, 2, NCLS], F32, tag="h3w_sb")
            nc.sync.dma_start(w_sb[:, 0, :], wd["h3_w"][0:128, :])
            nc.sync.dma_start(w_sb[:, 1, :], wd["h3_w"][128:256, :])
            t_sb = h3wp.tile([1, NCLS], F32, tag="h3t_sb")
            nc.sync.dma_start(t_sb[:], wd["h3_t"][:, :])
            for qt in range(NT_OWN):
                ps = h3ps.tile([128, NCLS], F32)
                for kc in range(2):
                    nc.tensor.matmul(
                        ps[:],
                        h2T[kc][:, qt * 128 : (qt + 1) * 128],
                        w_sb[:, kc, :],
                        start=(kc == 0),
                        stop=False,
                    )
                nc.tensor.matmul(
                    ps[:], ones512[:, 0:128], t_sb[:, :], start=False, stop=True
                )
                osb = h3op.tile([128, NCLS], F32, tag="osb")
                nc.scalar.activation(osb[:], ps[:], AF.Copy)
                nc.sync.dma_start(out_d[qt * 128 : (qt + 1) * 128, :], osb[:])

    split_multiwait(nc)
    return nc


def kernel(xyz, params, trace=False):
    import os

    import concourse.bass_utils as bass_utils

    xyz = np.asarray(xyz, np.float32)
    if "prog" not in _PROG_CACHE:
        _PROG_CACHE["prog"] = build_program()
    nc = _PROG_CACHE["prog"]

    wshared = host_weights(params)
    core_ids = list(range(8))
    in_maps = []
    for c in core_ids:
        m = dict(wshared)
        m.update(host_core_inputs(xyz, c))
        in_maps.append(m)
    res = bass_utils.run_bass_kernel_spmd(
        nc, in_maps, core_ids, trace=trace or bool(os.environ.get("KTRACE"))
    )
    _PROG_CACHE["last_res"] = res
    out = np.zeros((B, N, NCLS), np.float32)
    for c in core_ids:
        b, h = c // 2, c % 2
        out[b, h * NHALF : (h + 1) * NHALF, :] = res.results[c]["out"]
    return out
